# revision 1
# baseline (speedup 1.0000x reference)
"""Trainium2 Bass kernel for a GCN layer (gnn_message_passing).

Reference computation (per batch b):
    node_msg = h @ Wn_w.T + Wn_b                  # (N, OD)
    edge_msg = edge_feat @ We_w.T + We_b          # (N, N, OD)
    self_msg = h @ Ws_w.T + Ws_b                  # (N, OD)
    deg      = adj.sum(-1)                        # (N,)
    agg      = node_msg * deg + einsum('ij,ijo->io', adj, edge_msg)
    out      = relu(agg / clip(deg, 1) + self_msg)

Key algebraic rewrite: the (N,N,OD) edge_msg is never materialized.
    einsum('ij,ijo->io', adj, edge_feat @ We.T + We_b)
      = (einsum('ij,ije->ie', adj, edge_feat)) @ We.T + deg * We_b
so the dominant work is the adj-masked reduction of edge_feat over the
source-node axis j, producing (N, ED), followed by a tiny 16->64 matmul.

Sharding: data-parallel over batch B=8 across the 8 NeuronCores (one
batch element per core); weights replicated.

Pipeline design (per core):
  - ef[b] streams over the SP HWDGE queue as (128p, j-seg, 16e) tiles;
    the stream is the ~50us backbone everything else hides behind.
    Blocks 1-2 are single whole-block DMAs (fewer per-op overheads);
    block 0 is j-halves (lets compute start at ~6.5us) and block 3 is
    half+quarter+quarter so the post-DMA tail is short.
  - adj DMAs go FIRST on the ACT queue; widen(int32->f32) and degree
    are fused in one ACT activation(Copy, accum_out) per block.
  - the masked reduce  msum[i,e] = sum_j ef[i,j,e] * adj_f[i,j]  uses
    scalar_tensor_tensor with accum_out, with the 16 e-channels split
    DVE (e 0..9) / Pool (e 10..15) so the two engines run concurrently
    and each ef segment is retired in ~3us.
  - node/self messages, biases and the deg scalings are all precomputed
    into a per-block SBUF tile `base = degr*(h Wn^T + bn + be) + h Ws^T
    + bs` while ef streams; the per-block tail is only
    transpose(msum) -> (16->64) matmul -> (r*pes)+base -> relu -> DMA.
"""

import os
import sys
from contextlib import ExitStack

import numpy as np


def _ensure_concourse():
    try:
        import concourse  # noqa: F401
        return
    except ImportError:
        pass
    for p in ("/opt/trn_rl_repo", "/root/.axon_site/_ro/trn_rl_repo"):
        if os.path.isdir(p) and p not in sys.path:
            sys.path.insert(0, p)
            try:
                import concourse  # noqa: F401
                return
            except ImportError:
                continue
    raise ImportError("cannot locate the concourse (bass) package")


_ensure_concourse()

import concourse.bacc as bacc  # noqa: E402
import concourse.bass as bass  # noqa: E402
import concourse.tile as tile  # noqa: E402
from concourse import mybir  # noqa: E402
from concourse.bass_utils import run_bass_kernel_spmd  # noqa: E402
from concourse.masks import make_identity  # noqa: E402

B, N, ND, ED, OD = 8, 512, 64, 16, 64
NCORES = 8
PB = 128           # destination-node block (SBUF partitions)
NBLK = N // PB     # 4

F32 = mybir.dt.float32
I32 = mybir.dt.int32

# ef DMA segmentation per block: (j0, jn) pieces.
SEGS = {
    0: [(0, 256), (256, 256)],
    1: [(0, 256), (256, 256)],
    2: [(0, 256), (256, 256)],
    3: [(0, 256), (256, 128), (384, 64), (448, 64)],
}
# e-channel split for the masked reduce: DVE does fused stt+accum; the
# rest go Pool (plain product; walrus rejects stt on GPSIMD) + ACT
# (activation Copy with accum_out reduces each product row).
DVE_E = list(range(0, 13))
PACT_E = list(range(13, 16))


def _row_ap(handle, n):
    """View a 1-D DRAM tensor (n,) as a (1, n) AP."""
    ap = handle.ap()
    return bass.AP(tensor=ap.tensor, offset=ap.offset, ap=[[0, 1], [1, n]])


def build_bass(repeat=1, mode="full", unroll=1):
    """Build the single-core Bass program (SPMD across 8 cores).

    repeat>1 wraps the whole body in an on-device For_i loop -- used only
    for timing (amortizes host dispatch overhead away).
    mode: "full" (default) | "dual" (ef stream alternates SP/ACT queues).
    """
    nc = bacc.Bacc(
        "TRN2",
        target_bir_lowering=False,
        debug=False,
        num_devices=NCORES,
    )

    h_d = nc.dram_tensor("h", [N, ND], F32, kind="ExternalInput")
    adj_d = nc.dram_tensor("adj", [N, N], I32, kind="ExternalInput")
    ef_d = nc.dram_tensor("edge_feat", [N, N, ED], F32, kind="ExternalInput")
    wn_d = nc.dram_tensor("Wn_w", [OD, ND], F32, kind="ExternalInput")
    wnb_d = nc.dram_tensor("Wn_b", [OD], F32, kind="ExternalInput")
    we_d = nc.dram_tensor("We_w", [OD, ED], F32, kind="ExternalInput")
    web_d = nc.dram_tensor("We_b", [OD], F32, kind="ExternalInput")
    ws_d = nc.dram_tensor("Ws_w", [OD, ND], F32, kind="ExternalInput")
    wsb_d = nc.dram_tensor("Ws_b", [OD], F32, kind="ExternalInput")
    out_d = nc.dram_tensor("out", [N, OD], F32, kind="ExternalOutput")

    h_ap = h_d.ap()
    adj_ap = adj_d.ap()
    ef_ap = ef_d.ap()
    out_ap = out_d.ap()

    with tile.TileContext(nc) as tc, ExitStack() as ctx:
        consts = ctx.enter_context(tc.tile_pool(name="consts", bufs=1))
        efp = ctx.enter_context(tc.tile_pool(name="efp", bufs=1))
        adjp = ctx.enter_context(tc.tile_pool(name="adjp", bufs=1))
        work = ctx.enter_context(tc.tile_pool(name="work", bufs=1))
        outp = ctx.enter_context(tc.tile_pool(name="outp", bufs=2))
        prodp = ctx.enter_context(tc.tile_pool(name="prodp", bufs=2))
        pset = ctx.enter_context(tc.tile_pool(name="pset", bufs=2, space="PSUM"))
        pmm = ctx.enter_context(tc.tile_pool(name="pmm", bufs=2, space="PSUM"))
        pep = ctx.enter_context(tc.tile_pool(name="pep", bufs=2, space="PSUM"))

        def emit_body():
            ident = consts.tile([128, 128], F32)
            make_identity(nc, ident)

            # ---- queue head: adj DMAs + fused widen/degree ----
            # full: adj rides the ACT queue ahead of weights; dual: the SP
            # queue head (ACT is busy early and SP's ef start shifts ~3us,
            # which dual mode absorbs across two queues).
            adj_eng = nc.sync if mode == "dual" else nc.scalar
            adj_fs, degs = [], []
            for ib in range(NBLK):
                i0 = ib * PB
                adj_i = adjp.tile([PB, N], I32, tag=f"adji{ib}")
                adj_eng.dma_start(out=adj_i, in_=adj_ap[i0 : i0 + PB, :])
                adj_f = adjp.tile([PB, N], F32, tag=f"adjf{ib}")
                deg = work.tile([PB, 1], F32, tag=f"deg{ib}")
                nc.scalar.activation(
                    out=adj_f,
                    in_=adj_i,
                    func=mybir.ActivationFunctionType.Copy,
                    accum_out=deg,
                )
                adj_fs.append(adj_f)
                degs.append(deg)

            # degc = max(deg,1) [Pool]; r = 1/degc [DVE]; degr = deg*r [DVE]
            rs, degrs = [], []
            for ib in range(NBLK):
                degc = work.tile([PB, 1], F32, tag=f"degc{ib}")
                nc.gpsimd.tensor_scalar_max(degc, degs[ib], 1.0)
                r = work.tile([PB, 1], F32, tag=f"r{ib}")
                nc.vector.reciprocal(r, degc)
                degr = work.tile([PB, 1], F32, tag=f"degr{ib}")
                nc.vector.tensor_mul(degr, degs[ib], r)
                rs.append(r)
                degrs.append(degr)

            # ---- weights: transpose on PE; biases folded as extra row ----
            wn_sb = consts.tile([OD, ND], F32, tag="wload")
            nc.scalar.dma_start(out=wn_sb, in_=wn_d.ap())
            ws_sb = consts.tile([OD, ND], F32, tag="wload2")
            nc.scalar.dma_start(out=ws_sb, in_=ws_d.ap())
            we_sb = consts.tile([OD, ED], F32, tag="wload3")
            nc.scalar.dma_start(out=we_sb, in_=we_d.ap())

            rhs_n = consts.tile([ND + 1, OD], F32)
            rhs_s = consts.tile([ND + 1, OD], F32)
            weT = consts.tile([ED, OD], F32)

            # (PSUM->SBUF copies ride DVE, keeping the ACT queue free for
            # its DMA + widen + reduce duties.)
            pw = pset.tile([ND, OD], F32, tag="t")
            nc.tensor.transpose(pw, wn_sb, ident[:ND, :OD])
            nc.vector.tensor_copy(out=rhs_n[0:ND, :], in_=pw)
            pw2 = pset.tile([ND, OD], F32, tag="t")
            nc.tensor.transpose(pw2, ws_sb, ident[:ND, :OD])
            nc.vector.tensor_copy(out=rhs_s[0:ND, :], in_=pw2)
            pw3 = pset.tile([ED, OD], F32, tag="t")
            nc.tensor.transpose(pw3, we_sb, ident[:ND, :OD])
            nc.vector.tensor_copy(out=weT, in_=pw3)

            bias_n = consts.tile([1, OD], F32)
            nc.scalar.dma_start(out=bias_n, in_=_row_ap(wnb_d, OD))
            bias_e = consts.tile([1, OD], F32)
            nc.scalar.dma_start(out=bias_e, in_=_row_ap(web_d, OD))
            nc.vector.tensor_add(rhs_n[ND : ND + 1, :], bias_n, bias_e)
            nc.scalar.dma_start(out=rhs_s[ND : ND + 1, :], in_=_row_ap(wsb_d, OD))

            # ---- h^T with an appended ones-row: (65, 512) ----
            hT = consts.tile([ND + 1, N], F32)
            nc.vector.memset(hT[ND : ND + 1, :], 1.0)
            for ib in range(NBLK):
                h_sb = work.tile([PB, ND], F32, tag=f"hload{ib}")
                nc.scalar.dma_start(out=h_sb, in_=h_ap[ib * PB : (ib + 1) * PB, :])
                ph = pset.tile([ND, PB], F32, tag="t")
                nc.tensor.transpose(ph, h_sb, ident)
                nc.vector.tensor_copy(out=hT[0:ND, ib * PB : (ib + 1) * PB], in_=ph)

            # ---- precompute base = degr*(node+biases) + self  (off path) ----
            bases = []
            for ib in range(NBLK):
                i0 = ib * PB
                pn = pmm.tile([PB, OD], F32, tag="pn")
                nc.tensor.matmul(
                    pn, lhsT=hT[:, i0 : i0 + PB], rhs=rhs_n, start=True, stop=True
                )
                hs = pmm.tile([PB, OD], F32, tag="hs")
                nc.tensor.matmul(
                    hs, lhsT=hT[:, i0 : i0 + PB], rhs=rhs_s, start=True, stop=True
                )
                # (Pool/GPSIMD may not touch PSUM on HW -- keep these on DVE.)
                an = work.tile([PB, OD], F32, tag=f"an{ib}")
                nc.vector.tensor_scalar_mul(an, pn, degrs[ib])
                base = work.tile([PB, OD], F32, tag=f"base{ib}")
                nc.vector.tensor_add(base, an, hs)
                bases.append(base)

            # ---- ef stream: emit every segment DMA up front so both HWDGE
            # queues run the backbone unimpeded ----
            scr_d = work.tile([PB, N], F32, tag="scrd")
            scr_p = work.tile([PB, N], F32, tag="scrp")
            ef_ts = []
            for ib in range(NBLK):
                i0 = ib * PB
                ef_t = efp.tile([PB, N, ED], F32, tag=f"ef{ib}")
                for si, (j0, jn) in enumerate(SEGS[ib]):
                    eng = nc.sync
                    if mode == "dual" and si % 2 == 1:
                        eng = nc.scalar
                    eng.dma_start(
                        out=ef_t[:, j0 : j0 + jn, :],
                        in_=ef_ap[i0 : i0 + PB, j0 : j0 + jn, :],
                    )
                ef_ts.append(ef_t)

            # ---- per-block: split masked reduce, projection, combine ----
            for ib in range(NBLK):
                i0 = ib * PB
                adj_f = adj_fs[ib]
                segs = SEGS[ib]
                ef_t = ef_ts[ib]

                partials = []
                for si, (j0, jn) in enumerate(segs):
                    ms_p = work.tile([PB, ED], F32, tag=f"ms{ib}_{si}")
                    for e in DVE_E:
                        nc.vector.scalar_tensor_tensor(
                            out=scr_d[:, 0:jn],
                            in0=ef_t[:, j0 : j0 + jn, e],
                            scalar=1.0,
                            in1=adj_f[:, j0 : j0 + jn],
                            op0=mybir.AluOpType.bypass,
                            op1=mybir.AluOpType.mult,
                            accum_out=ms_p[:, e : e + 1],
                        )
                    # walrus can't lower stt on GPSIMD: Pool writes the plain
                    # masked product per channel, ACT reduces it via
                    # activation(Copy, accum_out).
                    prod = prodp.tile([PB, len(PACT_E), 256], F32, tag="prod")
                    for idx, e in enumerate(PACT_E):
                        nc.gpsimd.tensor_tensor(
                            out=prod[:, idx, 0:jn],
                            in0=ef_t[:, j0 : j0 + jn, e],
                            in1=adj_f[:, j0 : j0 + jn],
                            op=mybir.AluOpType.mult,
                        )
                    for idx, e in enumerate(PACT_E):
                        nc.scalar.activation(
                            out=scr_p[:, 0:jn],
                            in_=prod[:, idx, 0:jn],
                            func=mybir.ActivationFunctionType.Copy,
                            accum_out=ms_p[:, e : e + 1],
                        )
                    partials.append(ms_p)

                msum = partials[0]
                for si in range(1, len(partials)):
                    nxt = work.tile([PB, ED], F32, tag=f"msum{ib}_{si}")
                    nc.vector.tensor_add(nxt, msum, partials[si])
                    msum = nxt

                # (128,16) -> (16,128), project with We^T, combine, relu
                pm = pset.tile([ED, PB], F32, tag="t")
                nc.tensor.transpose(pm, msum, ident)
                msT = work.tile([ED, PB], F32, tag=f"msT{ib}")
                nc.scalar.copy(out=msT, in_=pm)
                pes = pep.tile([PB, OD], F32, tag="pes")
                nc.tensor.matmul(pes, lhsT=msT, rhs=weT, start=True, stop=True)

                ob = outp.tile([PB, OD], F32, tag="ob")
                nc.vector.scalar_tensor_tensor(
                    out=ob,
                    in0=pes,
                    scalar=rs[ib],
                    in1=bases[ib],
                    op0=mybir.AluOpType.mult,
                    op1=mybir.AluOpType.add,
                )
                nc.vector.tensor_scalar_max(ob, ob, 0.0)
                nc.scalar.dma_start(out=out_ap[i0 : i0 + PB, :], in_=ob)

        if repeat == 1:
            for _ in range(unroll):
                emit_body()
        else:
            with tc.For_i(0, repeat, 1):
                for _ in range(unroll):
                    emit_body()

    nc.compile()
    return nc


_NC_CACHE = None


def _get_nc():
    global _NC_CACHE
    if _NC_CACHE is None:
        _NC_CACHE = build_bass()
    return _NC_CACHE


def make_in_maps(inputs):
    w = {
        k: np.ascontiguousarray(np.asarray(inputs[k], dtype=np.float32))
        for k in ("Wn_w", "Wn_b", "We_w", "We_b", "Ws_w", "Ws_b")
    }
    h = np.asarray(inputs["h"], dtype=np.float32)
    adj = np.asarray(inputs["adj"], dtype=np.int32)
    ef = np.asarray(inputs["edge_feat"], dtype=np.float32)
    in_maps = []
    for c in range(NCORES):
        m = dict(w)
        m["h"] = np.ascontiguousarray(h[c])
        m["adj"] = np.ascontiguousarray(adj[c])
        m["edge_feat"] = np.ascontiguousarray(ef[c])
        in_maps.append(m)
    return in_maps


def run(inputs, trace=False):
    """Run on hardware; returns (full_output, BassKernelResults)."""
    nc = _get_nc()
    res = run_bass_kernel_spmd(nc, make_in_maps(inputs), list(range(NCORES)), trace=trace)
    out = np.stack(
        [np.asarray(res.results[c]["out"]) for c in range(NCORES)], axis=0
    ).astype(np.float32)
    return out, res


def kernel(**inputs):
    out, _ = run(inputs)
    return out

